# revision 14
# baseline (speedup 1.0000x reference)
"""Trainium2 Bass kernel for a 2-layer LIF spiking net (snnTorch Leaky,
subtract reset), batch-sharded across 8 NeuronCores.

Reference semantics (per step, both layers):
    reset = (mem > 1).float()            # == spk from previous step
    mem   = beta*mem + cur - reset
    spk   = (mem > 1).float()

Stage 1 (hidden layer): cur1 = x@w1.T + b1 is constant over time.
Per-core state held in SBUF in [h, b] layout (h on partitions), using a
negated/offset state z = -mem - 1/2 so the whole step is:
    PE  : w'   = (-beta*I) @ z + I @ cur1b          (PSUM; cur1b = cur1 + (1-beta)/2)
    DVE : z'   = (spk_prev * 1.0) - w'              (one fused scalar_tensor_tensor)
    ACT : spk  = sigmoid((-BIG)*z' - 1.5*BIG)       (exact 0/1: saturated sigmoid)
Stage 2 (output layer) in [b, o] packed layout (b%128 on partitions):
    PE  : cur2 = sum_h spk1^T-tiles @ w2.T-tiles + ones@b2   (PSUM accumulate)
    DVE : w2s  = (m2 * beta) + cur2
    GPS : m2   = w2s - spk2_prev ; spk2 = (m2 > 1)

Wall time is dominated by the axon tunnel (~56 MB/s per client d2h,
~120 MB/s aggregate), so the kernel minimizes and parallelizes traffic:
  * outputs shipped small: mem as f16, spikes Horner-packed 16-per-f32
    (exact integers < 2^16); host decodes with one unpackbits pass
  * donated output buffers live on-device (no zero-upload per call)
  * N worker processes each own a PJRT client for 8/N cores, fetching
    concurrently over separate tunnel connections into shared memory
"""
import os
import sys

for _p in ("/root/.axon_site/_ro/trn_rl_repo", "/opt/trn_rl_repo"):
    if _p not in sys.path:
        sys.path.append(_p)

import numpy as np

P = 128
T = 32
B_FULL, NI, NH, NO = 16384, 256, 512, 128
N_CORES = 8
BC = B_FULL // N_CORES          # 2048 batch rows per core
HB = NH // P                    # 4 hidden-layer partition tiles
IB = NI // P                    # 2 input partition tiles
BT = BC // P                    # 16 batch tiles of 128
NW = NO // 16                   # 8 packed 16-bit spike words per output row
BETA = 0.95
BIG = float(2.0 ** 100)

NPROC = int(os.environ.get("BASSK_NPROC", "4"))
assert N_CORES % NPROC == 0
G = N_CORES // NPROC            # cores per worker


# ---------------------------------------------------------------------------
# Bass kernel (per-core program; identical for every core)
# ---------------------------------------------------------------------------

def _build(t_steps=T, bc=BC):
    import concourse.bacc as bacc
    import concourse.tile as tile
    from concourse import mybir

    f32 = mybir.dt.float32
    f16 = mybir.dt.float16
    Alu = mybir.AluOpType
    Act = mybir.ActivationFunctionType
    bt = bc // P

    nc = bacc.Bacc(None, target_bir_lowering=False, debug=False)
    xT_d = nc.declare_dram_parameter("xT", [NI, bc], f32, isOutput=False)
    w1t_d = nc.declare_dram_parameter("w1t", [NI, NH], f32, isOutput=False)
    w2t_d = nc.declare_dram_parameter("w2t", [NH, NO], f32, isOutput=False)
    b1e_d = nc.declare_dram_parameter("b1e", [1, NH], f32, isOutput=False)
    b2_d = nc.declare_dram_parameter("b2", [1, 4 * NO], f32, isOutput=False)
    spkp_d = nc.declare_dram_parameter("spkp", [t_steps, bc, NW], f32, isOutput=True)
    mem_d = nc.declare_dram_parameter("mem", [t_steps, bc, NO], f16, isOutput=True)

    with tile.TileContext(nc) as tc:
        with (
            tc.tile_pool(name="const", bufs=1) as constp,
            tc.tile_pool(name="state", bufs=1) as statep,
            tc.tile_pool(name="spk1p", bufs=2) as spk1p,
            tc.tile_pool(name="work", bufs=1) as workp,
            tc.tile_pool(name="outp", bufs=2) as outp,
            tc.tile_pool(name="memh", bufs=2) as memhp,
            tc.tile_pool(name="pkp", bufs=3) as pkp,
            tc.tile_pool(name="pw", bufs=2, space="PSUM") as pwp,  # half tiles: 2x2 banks
            tc.tile_pool(name="p2", bufs=1, space="PSUM") as p2p,
        ):
            # ---- constants ----
            w1t_sb = constp.tile([P, IB, NH], f32)
            nc.sync.dma_start(w1t_sb, w1t_d[:].rearrange("(ib p) h -> p ib h", p=P))
            w2t_sb = constp.tile([P, HB, NO], f32)
            nc.sync.dma_start(w2t_sb, w2t_d[:].rearrange("(hb p) o -> p hb o", p=P))
            b1e_sb = constp.tile([P, HB], f32)
            nc.sync.dma_start(b1e_sb, b1e_d[:].rearrange("1 (hb p) -> p hb", p=P))
            b2_sb = constp.tile([1, 4 * NO], f32)
            nc.sync.dma_start(b2_sb, b2_d[:])
            ones_sb = constp.tile([1, P], f32)
            nc.vector.memset(ones_sb, 1.0)
            bigbias = constp.tile([P, 1], f32)
            nc.vector.memset(bigbias, -1.0 * BIG)
            ident = constp.tile([P, P], f32)
            nc.gpsimd.memset(ident, 0.0)
            nc.gpsimd.affine_select(
                out=ident[:], in_=ident[:], compare_op=Alu.not_equal,
                fill=1.0, base=0, pattern=[[-1, P]], channel_multiplier=1,
            )
            nbi = constp.tile([P, P], f32)
            nc.gpsimd.memset(nbi, 0.0)
            nc.gpsimd.affine_select(
                out=nbi[:], in_=nbi[:], compare_op=Alu.not_equal,
                fill=BETA, base=0, pattern=[[-1, P]], channel_multiplier=1,
            )

            # ---- prologue: cur1b = x@w1.T + b1e in [h, b] layout ----
            cur1b = constp.tile([P, HB, bc], f32)
            with tc.tile_pool(name="xtp", bufs=1) as xtp:
                xT_sb = xtp.tile([P, IB, bc], f32)
                nc.sync.dma_start(xT_sb, xT_d[:].rearrange("(ib p) b -> p ib b", p=P))
                for hb in range(HB):
                    pps = p2p.tile([P, bc], f32, tag="cur2")
                    for ch in range(bc // 512):
                        sl = slice(ch * 512, (ch + 1) * 512)
                        for ib in range(IB):
                            nc.tensor.matmul(
                                pps[:, sl],
                                w1t_sb[:, ib, hb * P:(hb + 1) * P],
                                xT_sb[:, ib, sl],
                                start=(ib == 0),
                                stop=(ib == IB - 1),
                            )
                    nc.scalar.activation(
                        cur1b[:, hb], pps, Act.Identity,
                        bias=b1e_sb[:, hb:hb + 1], scale=1.0,
                    )

            # ---- states ----
            z_tiles = []
            for hb in range(HB):
                zt = statep.tile([P, bc], f32, tag=f"z_{hb}")
                nc.vector.memset(zt, 0.0)
                z_tiles.append(zt)
            m2_sb = statep.tile([P, bt * NO], f32)
            nc.gpsimd.memset(m2_sb, 0.0)
            spk1_prev = []
            for hb in range(HB):
                s = spk1p.tile([P, bc], f32, tag=f"spk1_{hb}")
                nc.scalar.mul(s, z_tiles[hb], 0.0)  # zeros via ACT (keeps DVE free)
                spk1_prev.append(s)
            spk2_prev = outp.tile([P, bt * NO], f32, tag="spk2")
            nc.scalar.mul(spk2_prev, m2_sb, 0.0)

            # ---- time loop (fully unrolled) ----
            for t in range(t_steps):
                half = bc // 2
                spk1_cur = []
                for hb in range(HB):
                    for hf in range(2):
                        wp = pwp.tile([P, half], f32, tag="w1")
                        for ch in range(half // 512):
                            sl = slice(hf * half + ch * 512,
                                       hf * half + (ch + 1) * 512)
                            wsl = slice(ch * 512, (ch + 1) * 512)
                            nc.tensor.matmul(
                                wp[:, wsl], nbi[:], z_tiles[hb][:, sl],
                                start=True, stop=False,
                            )
                        for ch in range(half // 512):
                            sl = slice(hf * half + ch * 512,
                                       hf * half + (ch + 1) * 512)
                            wsl = slice(ch * 512, (ch + 1) * 512)
                            nc.tensor.matmul(
                                wp[:, wsl], ident[:], cur1b[:, hb, sl],
                                start=False, stop=True,
                            )
                        hsl = slice(hf * half, (hf + 1) * half)
                        # m1' = (spk_prev * -1) + w   (= w - spk_prev)
                        nc.vector.scalar_tensor_tensor(
                            z_tiles[hb][:, hsl], spk1_prev[hb][:, hsl], -1.0, wp,
                            Alu.mult, Alu.add
                        )
                    s = spk1p.tile([P, bc], f32, tag=f"spk1_{hb}")
                    nc.scalar.activation(
                        s, z_tiles[hb], Act.Sigmoid, bias=bigbias[:], scale=BIG
                    )
                    spk1_cur.append(s)

                # stage-2 matmuls: cur2 in [b, o] packed PSUM.
                # start=True clears the whole PSUM bank, so each bank leads
                # with one K=1 N=512 matmul broadcasting b2 across the bank;
                # all per-region spike matmuls then accumulate onto it.
                ps2 = p2p.tile([P, bt * NO], f32, tag="cur2")
                for bank in range(bt * NO // 512):
                    bsl2 = slice(bank * 512, (bank + 1) * 512)
                    nc.tensor.matmul(
                        ps2[:, bsl2], ones_sb, b2_sb, start=True, stop=False,
                        skip_group_check=True,
                    )
                    for j in range(512 // NO):
                        ib2 = bank * (512 // NO) + j
                        osl = slice(ib2 * NO, (ib2 + 1) * NO)
                        bsl = slice(ib2 * P, (ib2 + 1) * P)
                        for hb in range(HB):
                            nc.tensor.matmul(
                                ps2[:, osl], spk1_cur[hb][:, bsl], w2t_sb[:, hb],
                                start=False,
                                stop=(j == 512 // NO - 1 and hb == HB - 1),
                                skip_group_check=True,
                            )

                # stage-2 LIF
                w2s = workp.tile([P, bt * NO], f32, tag="w2s")
                nc.vector.scalar_tensor_tensor(
                    w2s, m2_sb, BETA, ps2, Alu.mult, Alu.add
                )
                nc.gpsimd.tensor_tensor(m2_sb, w2s, spk2_prev, Alu.subtract)
                spk2 = outp.tile([P, bt * NO], f32, tag="spk2")
                nc.gpsimd.tensor_scalar(spk2, m2_sb, 1.0, None, Alu.is_gt)

                # mem output: one f32->f16 cast, DMA in final [b, o] layout
                m2h = memhp.tile([P, bt * NO], f16, tag="m2h")
                nc.scalar.copy(m2h, m2_sb)
                nc.sync.dma_start(
                    mem_d[t].rearrange("(ib2 p) o -> p ib2 o", p=P),
                    m2h[:].rearrange("p (ib2 o) -> p ib2 o", o=NO),
                )
                # spike output: Horner-pack 16 adjacent o-bits into one f32
                # word (exact: integers < 2^16), 8 words per output row
                sv = spk2[:].rearrange("p (c j) -> p c j", j=16)
                pk = pkp.tile([P, bt * NW], f32, tag="pk")
                nc.vector.scalar_tensor_tensor(
                    pk, sv[:, :, 15], 2.0, sv[:, :, 14], Alu.mult, Alu.add
                )
                for j in range(13, -1, -1):
                    nc.vector.scalar_tensor_tensor(
                        pk, pk, 2.0, sv[:, :, j], Alu.mult, Alu.add
                    )
                nc.sync.dma_start(
                    spkp_d[t].rearrange("(ib2 p) k -> p ib2 k", p=P),
                    pk[:].rearrange("p (ib2 k) -> p ib2 k", k=NW),
                )

                spk1_prev = spk1_cur
                spk2_prev = spk2

    nc.finalize()
    return nc


# ---------------------------------------------------------------------------
# Worker: owns one PJRT client, drives G cores, decodes into shared memory
# ---------------------------------------------------------------------------

_WRT: dict = {}


def _worker_runtime(rank):
    if _WRT:
        return _WRT

    import jax
    from jax.experimental.shard_map import shard_map
    from jax.sharding import Mesh, NamedSharding, PartitionSpec
    from concourse import bass2jax, mybir

    # Disk cache for the BIR->NEFF compile (saves ~50s per worker/process).
    # The BIR bytes carry per-mesh metadata, but every worker compiles the
    # SAME per-core program (DRAM tensor names are ours and deterministic),
    # so key on the kernel-builder source instead; end-to-end rel-err
    # still validates the result.
    import hashlib
    import inspect
    import shutil
    _orig_cbk = bass2jax.compile_bir_kernel
    _src_key = hashlib.sha256(
        (inspect.getsource(_build) + f"|{T}|{BC}|v1").encode()
    ).hexdigest()[:24]

    def _cached_cbk(bir_json, tmpdir, neff_name="file.neff"):
        cdir = "/tmp/bassk_neffcache"
        os.makedirs(cdir, exist_ok=True)
        cpath = os.path.join(cdir, f"{_src_key}_{neff_name}")
        if os.path.exists(cpath):
            dst = os.path.join(tmpdir, neff_name)
            shutil.copy(cpath, dst)
            return dst
        p = _orig_cbk(bir_json, tmpdir, neff_name)
        try:
            shutil.copy(p, cpath + f".tmp{os.getpid()}")
            os.replace(cpath + f".tmp{os.getpid()}", cpath)
        except Exception:
            pass
        return p

    bass2jax.compile_bir_kernel = _cached_cbk

    bass2jax.install_neuronx_cc_hook()
    nc = _build()

    partition_name = (
        nc.partition_id_tensor.name if nc.partition_id_tensor is not None else None
    )
    in_names: list[str] = []
    out_names: list[str] = []
    out_avals: list = []
    for alloc in nc.m.functions[0].allocations:
        if not isinstance(alloc, mybir.MemoryLocationSet):
            continue
        name = alloc.memorylocations[0].name
        if alloc.kind == "ExternalInput":
            if name != partition_name:
                in_names.append(name)
        elif alloc.kind == "ExternalOutput":
            out_names.append(name)
            out_avals.append(
                jax.core.ShapedArray(
                    tuple(alloc.tensor_shape), mybir.dt.np(alloc.dtype)
                )
            )
    n_params = len(in_names)
    n_outs = len(out_avals)
    all_in_names = in_names + out_names
    if partition_name is not None:
        all_in_names = all_in_names + [partition_name]

    def _body(*args):
        operands = list(args)
        if partition_name is not None:
            operands.append(bass2jax.partition_id_tensor())
        outs = bass2jax._bass_exec_p.bind(
            *operands,
            out_avals=tuple(out_avals),
            in_names=tuple(all_in_names),
            out_names=tuple(out_names),
            lowering_input_output_aliases=(),
            sim_require_finite=True,
            sim_require_nnan=True,
            nc=nc,
        )
        return tuple(outs)

    devices = jax.devices()[rank * G:(rank + 1) * G]
    assert len(devices) == G
    mesh = Mesh(np.asarray(devices), ("core",))
    in_specs = (PartitionSpec("core"),) * (n_params + n_outs)
    out_specs = (PartitionSpec("core"),) * n_outs
    donate = tuple(range(n_params, n_params + n_outs))
    sharded = jax.jit(
        shard_map(
            _body, mesh=mesh, in_specs=in_specs, out_specs=out_specs,
            check_rep=False,
        ),
        donate_argnums=donate,
        keep_unused=True,
    )

    out_shardings = tuple(
        NamedSharding(mesh, PartitionSpec("core")) for _ in range(n_outs)
    )
    global_out_shapes = [(G * a.shape[0], *a.shape[1:]) for a in out_avals]

    def make_zeros():
        import jax.numpy as jnp
        fn = jax.jit(
            lambda: tuple(
                jnp.zeros(s, a.dtype) for s, a in zip(global_out_shapes, out_avals)
            ),
            out_shardings=out_shardings,
        )
        return list(fn())

    _WRT.update(
        sharded=sharded, in_names=in_names, out_names=out_names,
        make_zeros=make_zeros, donate_bufs=None,
    )
    return _WRT


def _worker_run(rank, msg):
    import time
    from concurrent.futures import ThreadPoolExecutor
    from multiprocessing import shared_memory

    _dbg = bool(os.environ.get("BASSK_DEBUG"))
    _t0 = time.perf_counter()

    def _wmark(label):
        if _dbg:
            print(f"    [w{rank}] {label}: {time.perf_counter() - _t0:.3f}s",
                  file=sys.stderr, flush=True)

    rt = _worker_runtime(rank)
    _wmark("runtime")

    x_part = msg["x_part"]                     # [G*BC, NI] f32
    w1, b1, w2, b2 = msg["w1"], msg["b1"], msg["w2"], msg["b2"]

    xT_g = np.ascontiguousarray(
        x_part.reshape(G, BC, NI).transpose(0, 2, 1).reshape(G * NI, BC)
    )
    w1t = np.ascontiguousarray(w1.T)
    w2t = np.ascontiguousarray(w2.T)
    b1e = b1.reshape(1, NH).astype(np.float32)
    b2r = np.tile(b2, 4).reshape(1, 4 * NO)
    rep = {
        "xT": xT_g,
        "w1t": np.tile(w1t, (G, 1)),
        "w2t": np.tile(w2t, (G, 1)),
        "b1e": np.tile(b1e, (G, 1)),
        "b2": np.tile(b2r, (G, 1)),
    }
    concat_in = [rep[name] for name in rt["in_names"]]

    _wmark("prep")
    donate_bufs = rt["donate_bufs"]
    if donate_bufs is None:
        donate_bufs = rt["make_zeros"]()
    _wmark("donate")
    out_arrs = rt["sharded"](*concat_in, *donate_bufs)
    rt["donate_bufs"] = list(out_arrs)
    _wmark("dispatch")
    if _dbg:
        for a in out_arrs:
            a.block_until_ready()
        _wmark("exec ready")

    idx = {name: i for i, name in enumerate(rt["out_names"])}
    spkp_arr = out_arrs[idx["spkp"]]
    mem_arr = out_arrs[idx["mem"]]

    shm_spk = shared_memory.SharedMemory(name=msg["shm_spk"], track=False)
    shm_mem = shared_memory.SharedMemory(name=msg["shm_mem"], track=False)
    try:
        spk = np.ndarray((T, B_FULL, NO), dtype=np.float32, buffer=shm_spk.buf)
        mem = np.ndarray((T, B_FULL, NO), dtype=np.float32, buffer=shm_mem.buf)
        b0 = rank * G * BC                      # this worker's batch offset

        # whole-array fetches: one bulk transfer per output beats many
        # small concurrent per-shard RPCs through the tunnel relay
        def fetch_mem():
            buf = np.asarray(mem_arr)           # [G*T, BC, NO] f16
            _wmark("mem transfer")
            v = buf.reshape(G, T, BC, NO)
            for c in range(G):
                lo = b0 + c * BC
                mem[:, lo:lo + BC, :] = v[c]    # cast-assign pass
            return None

        def fetch_spk():
            buf = np.asarray(spkp_arr)          # [G*T, BC, NW] f32 words
            _wmark("spk transfer")
            w16 = buf.astype(np.uint16)         # exact integers < 2^16
            bits = np.unpackbits(
                w16.view(np.uint8), axis=-1, bitorder="little"
            ).reshape(G, T, BC, NO)
            for c in range(G):
                lo = b0 + c * BC
                spk[:, lo:lo + BC, :] = bits[c]
            return None

        with ThreadPoolExecutor(max_workers=2) as ex:
            futs = [ex.submit(fetch_mem), ex.submit(fetch_spk)]
            for f in futs:
                f.result()
        _wmark("fetch+decode")
    finally:
        shm_spk.close()
        shm_mem.close()
    return {"ok": True}


def _worker_main(rank, nproc):
    import pickle
    import struct
    import traceback

    # reserve the protocol channel, divert all other stdout to stderr
    proto_out = os.fdopen(os.dup(1), "wb")
    os.dup2(2, 1)
    stdin = os.fdopen(os.dup(0), "rb")

    def send(obj):
        payload = pickle.dumps(obj, protocol=pickle.HIGHEST_PROTOCOL)
        proto_out.write(struct.pack(">I", len(payload)))
        proto_out.write(payload)
        proto_out.flush()

    def recv():
        hdr = stdin.read(4)
        if len(hdr) < 4:
            return None
        n = struct.unpack(">I", hdr)[0]
        return pickle.loads(stdin.read(n))

    send({"ok": True, "pid": os.getpid()})
    while True:
        msg = recv()
        if msg is None or msg.get("cmd") == "exit":
            break
        try:
            if msg["cmd"] == "run":
                send(_worker_run(rank, msg))
            else:
                send({"err": f"unknown cmd {msg['cmd']}"})
        except BaseException:
            send({"err": traceback.format_exc()})


# ---------------------------------------------------------------------------
# Parent: spawn workers, dispatch, assemble shm-backed outputs
# ---------------------------------------------------------------------------

_PAR: dict = {}


def _ensure_workers():
    if _PAR.get("workers"):
        return _PAR["workers"]
    import atexit
    import pickle
    import struct
    import subprocess

    here = os.path.dirname(os.path.abspath(__file__))
    workers = []
    for r in range(NPROC):
        code = (
            f"import sys; sys.path.insert(0, {here!r}); "
            f"import kernel; kernel._worker_main({r}, {NPROC})"
        )
        logf = open(f"/tmp/bassk_worker{r}.log", "ab", buffering=0)
        p = subprocess.Popen(
            [sys.executable, "-c", code],
            stdin=subprocess.PIPE, stdout=subprocess.PIPE, stderr=logf,
        )
        workers.append(p)

    def send(p, obj):
        payload = pickle.dumps(obj, protocol=pickle.HIGHEST_PROTOCOL)
        p.stdin.write(struct.pack(">I", len(payload)))
        p.stdin.write(payload)
        p.stdin.flush()

    def recv(p):
        hdr = p.stdout.read(4)
        if len(hdr) < 4:
            raise RuntimeError(
                f"worker died (see /tmp/bassk_worker*.log): rc={p.poll()}"
            )
        n = struct.unpack(">I", hdr)[0]
        return pickle.loads(p.stdout.read(n))

    for p in workers:
        hello = recv(p)
        assert hello.get("ok"), hello

    def cleanup():
        for p in workers:
            try:
                send(p, {"cmd": "exit"})
            except Exception:
                pass
        for p in workers:
            try:
                p.wait(timeout=5)
            except Exception:
                p.kill()
        for shm in _PAR.get("shms", []):
            try:
                shm.close()
                shm.unlink()
            except Exception:
                pass

    atexit.register(cleanup)
    _PAR.update(workers=workers, send=send, recv=recv, shms=[], seq=0)
    return workers


def kernel(x, w1, b1, w2, b2, num_steps):
    import time
    from multiprocessing import shared_memory

    _dbg = bool(os.environ.get("BASSK_DEBUG"))
    _t0 = time.perf_counter()

    def _mark(label):
        if _dbg:
            print(f"    [k] {label}: {time.perf_counter() - _t0:.3f}s", flush=True)

    x = np.asarray(x, dtype=np.float32)
    w1 = np.asarray(w1, dtype=np.float32)
    b1 = np.asarray(b1, dtype=np.float32)
    w2 = np.asarray(w2, dtype=np.float32)
    b2 = np.asarray(b2, dtype=np.float32)
    t_steps = int(num_steps)
    assert x.shape == (B_FULL, NI) and t_steps == T

    workers = _ensure_workers()
    send, recv = _PAR["send"], _PAR["recv"]
    _mark("workers ready")

    nbytes = T * B_FULL * NO * 4
    seq = _PAR["seq"]
    _PAR["seq"] += 1
    shm_spk = shared_memory.SharedMemory(
        create=True, size=nbytes, name=f"bassk_{os.getpid()}_{seq}_s"
    )
    shm_mem = shared_memory.SharedMemory(
        create=True, size=nbytes, name=f"bassk_{os.getpid()}_{seq}_m"
    )
    # keep segments mapped for the life of the process: returned arrays
    # alias them, and the harness may hold results across later calls
    _PAR["shms"] += [shm_spk, shm_mem]

    def msg_for(r):
        return {
            "cmd": "run",
            "x_part": x[r * G * BC:(r + 1) * G * BC],
            "w1": w1, "b1": b1, "w2": w2, "b2": b2,
            "shm_spk": shm_spk.name, "shm_mem": shm_mem.name,
        }

    errs = []
    if seq == 0:
        # first call: serialize workers so their NEFF compiles (minutes,
        # single host CPU) don't contend or deadlock
        for r, p in enumerate(workers):
            send(p, msg_for(r))
            res = recv(p)
            if not res.get("ok"):
                errs.append(res.get("err"))
    else:
        for r, p in enumerate(workers):
            send(p, msg_for(r))
        _mark("dispatched")
        for p in workers:
            res = recv(p)
            if not res.get("ok"):
                errs.append(res.get("err"))
    if errs:
        raise RuntimeError("worker failure:\n" + "\n".join(errs))
    _mark("workers done")

    spk = np.ndarray((T, B_FULL, NO), dtype=np.float32, buffer=shm_spk.buf)
    mem = np.ndarray((T, B_FULL, NO), dtype=np.float32, buffer=shm_mem.buf)
    return spk, mem


# revision 17
# speedup vs baseline: 1.1073x; 1.1073x over previous
"""Trainium2 Bass kernel for a 2-layer LIF spiking net (snnTorch Leaky,
subtract reset), batch-sharded across 8 NeuronCores.

Reference semantics (per step, both layers):
    reset = (mem > 1).float()            # == spk from previous step
    mem   = beta*mem + cur - reset
    spk   = (mem > 1).float()

Stage 1 (hidden layer): cur1 = x@w1.T + b1 is constant over time.
Per-core state held in SBUF in [h, b] layout (h on partitions), using a
negated/offset state z = -mem - 1/2 so the whole step is:
    PE  : w'   = (-beta*I) @ z + I @ cur1b          (PSUM; cur1b = cur1 + (1-beta)/2)
    DVE : z'   = (spk_prev * 1.0) - w'              (one fused scalar_tensor_tensor)
    ACT : spk  = sigmoid((-BIG)*z' - 1.5*BIG)       (exact 0/1: saturated sigmoid)
Stage 2 (output layer) in [b, o] packed layout (b%128 on partitions):
    PE  : cur2 = sum_h spk1^T-tiles @ w2.T-tiles + ones@b2   (PSUM accumulate)
    DVE : w2s  = (m2 * beta) + cur2
    GPS : m2   = w2s - spk2_prev ; spk2 = (m2 > 1)

Wall time is dominated by the axon tunnel (~56 MB/s per client d2h,
~120 MB/s aggregate), so the kernel minimizes and parallelizes traffic:
  * outputs shipped small: mem as f16, spikes Horner-packed 16-per-f32
    (exact integers < 2^16); host decodes with one unpackbits pass
  * donated output buffers live on-device (no zero-upload per call)
  * N worker processes each own a PJRT client for 8/N cores, fetching
    concurrently over separate tunnel connections into shared memory
"""
import os
import sys

for _p in ("/root/.axon_site/_ro/trn_rl_repo", "/opt/trn_rl_repo"):
    if _p not in sys.path:
        sys.path.append(_p)

import numpy as np

P = 128
T = 32
B_FULL, NI, NH, NO = 16384, 256, 512, 128
N_CORES = 8
BC = B_FULL // N_CORES          # 2048 batch rows per core
HB = NH // P                    # 4 hidden-layer partition tiles
IB = NI // P                    # 2 input partition tiles
BT = BC // P                    # 16 batch tiles of 128
NW = NO // 16                   # 8 packed 16-bit spike words per output row
BETA = 0.95
BIG = float(2.0 ** 100)

NPROC = int(os.environ.get("BASSK_NPROC", "4"))
assert N_CORES % NPROC == 0
G = N_CORES // NPROC            # cores per worker


# ---------------------------------------------------------------------------
# Bass kernel (per-core program; identical for every core)
# ---------------------------------------------------------------------------

def _build(t_steps=T, bc=BC):
    import concourse.bacc as bacc
    import concourse.tile as tile
    from concourse import mybir

    f32 = mybir.dt.float32
    f16 = mybir.dt.float16
    Alu = mybir.AluOpType
    Act = mybir.ActivationFunctionType
    bt = bc // P

    nc = bacc.Bacc(None, target_bir_lowering=False, debug=False)
    xT_d = nc.declare_dram_parameter("xT", [NI, bc], f32, isOutput=False)
    w1t_d = nc.declare_dram_parameter("w1t", [NI, NH], f32, isOutput=False)
    w2t_d = nc.declare_dram_parameter("w2t", [NH, NO], f32, isOutput=False)
    b1e_d = nc.declare_dram_parameter("b1e", [1, NH], f32, isOutput=False)
    b2_d = nc.declare_dram_parameter("b2", [1, 4 * NO], f32, isOutput=False)
    spkp_d = nc.declare_dram_parameter("spkp", [t_steps, bc, NW], f32, isOutput=True)
    mem_d = nc.declare_dram_parameter("mem", [t_steps, bc, NO], f16, isOutput=True)

    with tile.TileContext(nc) as tc:
        with (
            tc.tile_pool(name="const", bufs=1) as constp,
            tc.tile_pool(name="state", bufs=1) as statep,
            tc.tile_pool(name="spk1p", bufs=2) as spk1p,
            tc.tile_pool(name="work", bufs=1) as workp,
            tc.tile_pool(name="outp", bufs=2) as outp,
            tc.tile_pool(name="memh", bufs=2) as memhp,
            tc.tile_pool(name="pkp", bufs=3) as pkp,
            tc.tile_pool(name="pw", bufs=2, space="PSUM") as pwp,  # half tiles: 2x2 banks
            tc.tile_pool(name="p2", bufs=1, space="PSUM") as p2p,
        ):
            # ---- constants ----
            w1t_sb = constp.tile([P, IB, NH], f32)
            nc.sync.dma_start(w1t_sb, w1t_d[:].rearrange("(ib p) h -> p ib h", p=P))
            w2t_sb = constp.tile([P, HB, NO], f32)
            nc.sync.dma_start(w2t_sb, w2t_d[:].rearrange("(hb p) o -> p hb o", p=P))
            b1e_sb = constp.tile([P, HB], f32)
            nc.sync.dma_start(b1e_sb, b1e_d[:].rearrange("1 (hb p) -> p hb", p=P))
            b2_sb = constp.tile([1, 4 * NO], f32)
            nc.sync.dma_start(b2_sb, b2_d[:])
            ones_sb = constp.tile([1, P], f32)
            nc.vector.memset(ones_sb, 1.0)
            bigbias = constp.tile([P, 1], f32)
            nc.vector.memset(bigbias, -1.0 * BIG)
            ident = constp.tile([P, P], f32)
            nc.gpsimd.memset(ident, 0.0)
            nc.gpsimd.affine_select(
                out=ident[:], in_=ident[:], compare_op=Alu.not_equal,
                fill=1.0, base=0, pattern=[[-1, P]], channel_multiplier=1,
            )
            nbi = constp.tile([P, P], f32)
            nc.gpsimd.memset(nbi, 0.0)
            nc.gpsimd.affine_select(
                out=nbi[:], in_=nbi[:], compare_op=Alu.not_equal,
                fill=BETA, base=0, pattern=[[-1, P]], channel_multiplier=1,
            )

            # ---- prologue: cur1b = x@w1.T + b1e in [h, b] layout ----
            cur1b = constp.tile([P, HB, bc], f32)
            with tc.tile_pool(name="xtp", bufs=1) as xtp:
                xT_sb = xtp.tile([P, IB, bc], f32)
                nc.sync.dma_start(xT_sb, xT_d[:].rearrange("(ib p) b -> p ib b", p=P))
                for hb in range(HB):
                    pps = p2p.tile([P, bc], f32, tag="cur2")
                    for ch in range(bc // 512):
                        sl = slice(ch * 512, (ch + 1) * 512)
                        for ib in range(IB):
                            nc.tensor.matmul(
                                pps[:, sl],
                                w1t_sb[:, ib, hb * P:(hb + 1) * P],
                                xT_sb[:, ib, sl],
                                start=(ib == 0),
                                stop=(ib == IB - 1),
                            )
                    nc.scalar.activation(
                        cur1b[:, hb], pps, Act.Identity,
                        bias=b1e_sb[:, hb:hb + 1], scale=1.0,
                    )

            # ---- states ----
            z_tiles = []
            for hb in range(HB):
                zt = statep.tile([P, bc], f32, tag=f"z_{hb}")
                nc.vector.memset(zt, 0.0)
                z_tiles.append(zt)
            m2_sb = statep.tile([P, bt * NO], f32)
            nc.gpsimd.memset(m2_sb, 0.0)
            spk1_prev = []
            for hb in range(HB):
                s = spk1p.tile([P, bc], f32, tag=f"spk1_{hb}")
                nc.scalar.mul(s, z_tiles[hb], 0.0)  # zeros via ACT (keeps DVE free)
                spk1_prev.append(s)
            spk2_prev = outp.tile([P, bt * NO], f32, tag="spk2")
            nc.scalar.mul(spk2_prev, m2_sb, 0.0)

            # ---- time loop (fully unrolled) ----
            for t in range(t_steps):
                half = bc // 2
                spk1_cur = []
                for hb in range(HB):
                    for hf in range(2):
                        wp = pwp.tile([P, half], f32, tag="w1")
                        for ch in range(half // 512):
                            sl = slice(hf * half + ch * 512,
                                       hf * half + (ch + 1) * 512)
                            wsl = slice(ch * 512, (ch + 1) * 512)
                            nc.tensor.matmul(
                                wp[:, wsl], nbi[:], z_tiles[hb][:, sl],
                                start=True, stop=False,
                            )
                        for ch in range(half // 512):
                            sl = slice(hf * half + ch * 512,
                                       hf * half + (ch + 1) * 512)
                            wsl = slice(ch * 512, (ch + 1) * 512)
                            nc.tensor.matmul(
                                wp[:, wsl], ident[:], cur1b[:, hb, sl],
                                start=False, stop=True,
                            )
                        hsl = slice(hf * half, (hf + 1) * half)
                        # m1' = (spk_prev * -1) + w   (= w - spk_prev)
                        nc.vector.scalar_tensor_tensor(
                            z_tiles[hb][:, hsl], spk1_prev[hb][:, hsl], -1.0, wp,
                            Alu.mult, Alu.add
                        )
                    s = spk1p.tile([P, bc], f32, tag=f"spk1_{hb}")
                    nc.scalar.activation(
                        s, z_tiles[hb], Act.Sigmoid, bias=bigbias[:], scale=BIG
                    )
                    spk1_cur.append(s)

                # stage-2 matmuls: cur2 in [b, o] packed PSUM.
                # start=True clears the whole PSUM bank, so each bank leads
                # with one K=1 N=512 matmul broadcasting b2 across the bank;
                # all per-region spike matmuls then accumulate onto it.
                ps2 = p2p.tile([P, bt * NO], f32, tag="cur2")
                for bank in range(bt * NO // 512):
                    bsl2 = slice(bank * 512, (bank + 1) * 512)
                    nc.tensor.matmul(
                        ps2[:, bsl2], ones_sb, b2_sb, start=True, stop=False,
                        skip_group_check=True,
                    )
                    for j in range(512 // NO):
                        ib2 = bank * (512 // NO) + j
                        osl = slice(ib2 * NO, (ib2 + 1) * NO)
                        bsl = slice(ib2 * P, (ib2 + 1) * P)
                        for hb in range(HB):
                            nc.tensor.matmul(
                                ps2[:, osl], spk1_cur[hb][:, bsl], w2t_sb[:, hb],
                                start=False,
                                stop=(j == 512 // NO - 1 and hb == HB - 1),
                                skip_group_check=True,
                            )

                # stage-2 LIF
                w2s = workp.tile([P, bt * NO], f32, tag="w2s")
                nc.vector.scalar_tensor_tensor(
                    w2s, m2_sb, BETA, ps2, Alu.mult, Alu.add
                )
                nc.gpsimd.tensor_tensor(m2_sb, w2s, spk2_prev, Alu.subtract)
                spk2 = outp.tile([P, bt * NO], f32, tag="spk2")
                nc.gpsimd.tensor_scalar(spk2, m2_sb, 1.0, None, Alu.is_gt)

                # mem output: one f32->f16 cast, DMA in final [b, o] layout
                m2h = memhp.tile([P, bt * NO], f16, tag="m2h")
                nc.scalar.copy(m2h, m2_sb)
                nc.sync.dma_start(
                    mem_d[t].rearrange("(ib2 p) o -> p ib2 o", p=P),
                    m2h[:].rearrange("p (ib2 o) -> p ib2 o", o=NO),
                )
                # spike output: Horner-pack 16 adjacent o-bits into one f32
                # word (exact: integers < 2^16), 8 words per output row
                sv = spk2[:].rearrange("p (c j) -> p c j", j=16)
                pk = pkp.tile([P, bt * NW], f32, tag="pk")
                nc.vector.scalar_tensor_tensor(
                    pk, sv[:, :, 15], 2.0, sv[:, :, 14], Alu.mult, Alu.add
                )
                for j in range(13, -1, -1):
                    nc.vector.scalar_tensor_tensor(
                        pk, pk, 2.0, sv[:, :, j], Alu.mult, Alu.add
                    )
                nc.sync.dma_start(
                    spkp_d[t].rearrange("(ib2 p) k -> p ib2 k", p=P),
                    pk[:].rearrange("p (ib2 k) -> p ib2 k", k=NW),
                )

                spk1_prev = spk1_cur
                spk2_prev = spk2

    nc.finalize()
    return nc


# ---------------------------------------------------------------------------
# Worker: owns one PJRT client, drives G cores, decodes into shared memory
# ---------------------------------------------------------------------------

_WRT: dict = {}


def _worker_runtime(rank):
    if _WRT:
        return _WRT

    import jax
    from jax.experimental.shard_map import shard_map
    from jax.sharding import Mesh, NamedSharding, PartitionSpec
    from concourse import bass2jax, mybir

    # Disk cache for the BIR->NEFF compile (saves ~50s per worker/process).
    # The BIR bytes carry per-mesh metadata, but every worker compiles the
    # SAME per-core program (DRAM tensor names are ours and deterministic),
    # so key on the kernel-builder source instead; end-to-end rel-err
    # still validates the result.
    import hashlib
    import inspect
    import shutil
    _orig_cbk = bass2jax.compile_bir_kernel
    _src_key = hashlib.sha256(
        (inspect.getsource(_build) + f"|{T}|{BC}|G{G}|v1").encode()
    ).hexdigest()[:24]

    def _cached_cbk(bir_json, tmpdir, neff_name="file.neff"):
        cdir = "/tmp/bassk_neffcache"
        os.makedirs(cdir, exist_ok=True)
        cpath = os.path.join(cdir, f"{_src_key}_{neff_name}")
        if os.path.exists(cpath):
            dst = os.path.join(tmpdir, neff_name)
            shutil.copy(cpath, dst)
            return dst
        p = _orig_cbk(bir_json, tmpdir, neff_name)
        try:
            shutil.copy(p, cpath + f".tmp{os.getpid()}")
            os.replace(cpath + f".tmp{os.getpid()}", cpath)
        except Exception:
            pass
        return p

    bass2jax.compile_bir_kernel = _cached_cbk

    bass2jax.install_neuronx_cc_hook()
    nc = _build()

    partition_name = (
        nc.partition_id_tensor.name if nc.partition_id_tensor is not None else None
    )
    in_names: list[str] = []
    out_names: list[str] = []
    out_avals: list = []
    for alloc in nc.m.functions[0].allocations:
        if not isinstance(alloc, mybir.MemoryLocationSet):
            continue
        name = alloc.memorylocations[0].name
        if alloc.kind == "ExternalInput":
            if name != partition_name:
                in_names.append(name)
        elif alloc.kind == "ExternalOutput":
            out_names.append(name)
            out_avals.append(
                jax.core.ShapedArray(
                    tuple(alloc.tensor_shape), mybir.dt.np(alloc.dtype)
                )
            )
    n_params = len(in_names)
    n_outs = len(out_avals)
    all_in_names = in_names + out_names
    if partition_name is not None:
        all_in_names = all_in_names + [partition_name]

    def _body(*args):
        operands = list(args)
        if partition_name is not None:
            operands.append(bass2jax.partition_id_tensor())
        outs = bass2jax._bass_exec_p.bind(
            *operands,
            out_avals=tuple(out_avals),
            in_names=tuple(all_in_names),
            out_names=tuple(out_names),
            lowering_input_output_aliases=(),
            sim_require_finite=True,
            sim_require_nnan=True,
            nc=nc,
        )
        return tuple(outs)

    devices = jax.devices()[rank * G:(rank + 1) * G]
    assert len(devices) == G
    mesh = Mesh(np.asarray(devices), ("core",))
    in_specs = (PartitionSpec("core"),) * (n_params + n_outs)
    out_specs = (PartitionSpec("core"),) * n_outs
    donate = tuple(range(n_params, n_params + n_outs))
    sharded = jax.jit(
        shard_map(
            _body, mesh=mesh, in_specs=in_specs, out_specs=out_specs,
            check_rep=False,
        ),
        donate_argnums=donate,
        keep_unused=True,
    )

    out_shardings = tuple(
        NamedSharding(mesh, PartitionSpec("core")) for _ in range(n_outs)
    )
    global_out_shapes = [(G * a.shape[0], *a.shape[1:]) for a in out_avals]

    def make_zeros():
        import jax.numpy as jnp
        fn = jax.jit(
            lambda: tuple(
                jnp.zeros(s, a.dtype) for s, a in zip(global_out_shapes, out_avals)
            ),
            out_shardings=out_shardings,
        )
        return list(fn())

    _WRT.update(
        sharded=sharded, in_names=in_names, out_names=out_names,
        make_zeros=make_zeros, donate_bufs=None,
    )
    return _WRT


def _worker_run(rank, msg):
    import time
    from multiprocessing import shared_memory

    _dbg = bool(os.environ.get("BASSK_DEBUG"))
    _t0 = time.perf_counter()

    def _wmark(label):
        if _dbg:
            print(f"    [w{rank}] {label}: {time.perf_counter() - _t0:.3f}s",
                  file=sys.stderr, flush=True)

    rt = _worker_runtime(rank)
    _wmark("runtime")

    x_part = msg["x_part"]                     # [G*BC, NI] f32
    w1, b1, w2, b2 = msg["w1"], msg["b1"], msg["w2"], msg["b2"]

    xT_g = np.ascontiguousarray(
        x_part.reshape(G, BC, NI).transpose(0, 2, 1).reshape(G * NI, BC)
    )
    w1t = np.ascontiguousarray(w1.T)
    w2t = np.ascontiguousarray(w2.T)
    b1e = b1.reshape(1, NH).astype(np.float32)
    b2r = np.tile(b2, 4).reshape(1, 4 * NO)
    rep = {
        "xT": xT_g,
        "w1t": np.tile(w1t, (G, 1)),
        "w2t": np.tile(w2t, (G, 1)),
        "b1e": np.tile(b1e, (G, 1)),
        "b2": np.tile(b2r, (G, 1)),
    }
    concat_in = [rep[name] for name in rt["in_names"]]

    _wmark("prep")
    donate_bufs = rt["donate_bufs"]
    if donate_bufs is None:
        donate_bufs = rt["make_zeros"]()
    _wmark("donate")
    out_arrs = rt["sharded"](*concat_in, *donate_bufs)
    rt["donate_bufs"] = list(out_arrs)
    _wmark("dispatch")
    if _dbg:
        for a in out_arrs:
            a.block_until_ready()
        _wmark("exec ready")

    idx = {name: i for i, name in enumerate(rt["out_names"])}
    spkp_arr = out_arrs[idx["spkp"]]
    mem_arr = out_arrs[idx["mem"]]

    shm_spk = shared_memory.SharedMemory(name=msg["shm_spk"], track=False)
    shm_mem = shared_memory.SharedMemory(name=msg["shm_mem"], track=False)
    try:
        spk = np.ndarray((T, B_FULL, NO), dtype=np.float32, buffer=shm_spk.buf)
        mem = np.ndarray((T, B_FULL, NO), dtype=np.float32, buffer=shm_mem.buf)
        b0 = rank * G * BC                      # this worker's batch offset

        # Transfers serialize per client anyway, and decode must not steal
        # CPU from the relay mid-transfer (single host core): fetch both
        # outputs back-to-back, decode strictly afterwards.
        sbuf = np.asarray(spkp_arr)             # [G*T, BC, NW] f32 words
        _wmark("spk transfer")
        mbuf = np.asarray(mem_arr)              # [G*T, BC, NO] f16
        _wmark("mem transfer")

        v = mbuf.reshape(G, T, BC, NO)
        for c in range(G):
            lo = b0 + c * BC
            mem[:, lo:lo + BC, :] = v[c]        # cast-assign pass
        w16 = sbuf.astype(np.uint16)            # exact integers < 2^16
        bits = np.unpackbits(
            w16.view(np.uint8), axis=-1, bitorder="little"
        ).reshape(G, T, BC, NO)
        for c in range(G):
            lo = b0 + c * BC
            spk[:, lo:lo + BC, :] = bits[c]
        _wmark("fetch+decode")
    finally:
        shm_spk.close()
        shm_mem.close()
    return {"ok": True}


def _worker_main(rank, nproc):
    import pickle
    import struct
    import traceback

    # reserve the protocol channel, divert all other stdout to stderr
    proto_out = os.fdopen(os.dup(1), "wb")
    os.dup2(2, 1)
    stdin = os.fdopen(os.dup(0), "rb")

    def send(obj):
        payload = pickle.dumps(obj, protocol=pickle.HIGHEST_PROTOCOL)
        proto_out.write(struct.pack(">I", len(payload)))
        proto_out.write(payload)
        proto_out.flush()

    def recv():
        hdr = stdin.read(4)
        if len(hdr) < 4:
            return None
        n = struct.unpack(">I", hdr)[0]
        return pickle.loads(stdin.read(n))

    send({"ok": True, "pid": os.getpid()})
    while True:
        msg = recv()
        if msg is None or msg.get("cmd") == "exit":
            break
        try:
            if msg["cmd"] == "run":
                send(_worker_run(rank, msg))
            else:
                send({"err": f"unknown cmd {msg['cmd']}"})
        except BaseException:
            send({"err": traceback.format_exc()})


# ---------------------------------------------------------------------------
# Parent: spawn workers, dispatch, assemble shm-backed outputs
# ---------------------------------------------------------------------------

_PAR: dict = {}


def _ensure_workers():
    if _PAR.get("workers"):
        return _PAR["workers"]
    import atexit
    import pickle
    import struct
    import subprocess

    here = os.path.dirname(os.path.abspath(__file__))
    workers = []
    for r in range(NPROC):
        code = (
            f"import sys; sys.path.insert(0, {here!r}); "
            f"import kernel; kernel._worker_main({r}, {NPROC})"
        )
        logf = open(f"/tmp/bassk_worker{r}.log", "ab", buffering=0)
        p = subprocess.Popen(
            [sys.executable, "-c", code],
            stdin=subprocess.PIPE, stdout=subprocess.PIPE, stderr=logf,
        )
        workers.append(p)

    def send(p, obj):
        payload = pickle.dumps(obj, protocol=pickle.HIGHEST_PROTOCOL)
        p.stdin.write(struct.pack(">I", len(payload)))
        p.stdin.write(payload)
        p.stdin.flush()

    def recv(p):
        hdr = p.stdout.read(4)
        if len(hdr) < 4:
            raise RuntimeError(
                f"worker died (see /tmp/bassk_worker*.log): rc={p.poll()}"
            )
        n = struct.unpack(">I", hdr)[0]
        return pickle.loads(p.stdout.read(n))

    for p in workers:
        hello = recv(p)
        assert hello.get("ok"), hello

    def cleanup():
        for p in workers:
            try:
                send(p, {"cmd": "exit"})
            except Exception:
                pass
        for p in workers:
            try:
                p.wait(timeout=5)
            except Exception:
                p.kill()
        for shm in _PAR.get("shms", []):
            try:
                shm.close()
                shm.unlink()
            except Exception:
                pass

    atexit.register(cleanup)
    _PAR.update(workers=workers, send=send, recv=recv, shms=[], seq=0)
    return workers


def kernel(x, w1, b1, w2, b2, num_steps):
    import time
    from multiprocessing import shared_memory

    _dbg = bool(os.environ.get("BASSK_DEBUG"))
    _t0 = time.perf_counter()

    def _mark(label):
        if _dbg:
            print(f"    [k] {label}: {time.perf_counter() - _t0:.3f}s", flush=True)

    x = np.asarray(x, dtype=np.float32)
    w1 = np.asarray(w1, dtype=np.float32)
    b1 = np.asarray(b1, dtype=np.float32)
    w2 = np.asarray(w2, dtype=np.float32)
    b2 = np.asarray(b2, dtype=np.float32)
    t_steps = int(num_steps)
    assert x.shape == (B_FULL, NI) and t_steps == T

    workers = _ensure_workers()
    send, recv = _PAR["send"], _PAR["recv"]
    _mark("workers ready")

    nbytes = T * B_FULL * NO * 4
    seq = _PAR["seq"]
    _PAR["seq"] += 1
    shm_spk = shared_memory.SharedMemory(
        create=True, size=nbytes, name=f"bassk_{os.getpid()}_{seq}_s"
    )
    shm_mem = shared_memory.SharedMemory(
        create=True, size=nbytes, name=f"bassk_{os.getpid()}_{seq}_m"
    )
    # keep segments mapped for the life of the process: returned arrays
    # alias them, and the harness may hold results across later calls
    _PAR["shms"] += [shm_spk, shm_mem]

    def msg_for(r):
        return {
            "cmd": "run",
            "x_part": x[r * G * BC:(r + 1) * G * BC],
            "w1": w1, "b1": b1, "w2": w2, "b2": b2,
            "shm_spk": shm_spk.name, "shm_mem": shm_mem.name,
        }

    errs = []
    if seq == 0:
        # first call: serialize workers so their NEFF compiles (minutes,
        # single host CPU) don't contend or deadlock
        for r, p in enumerate(workers):
            send(p, msg_for(r))
            res = recv(p)
            if not res.get("ok"):
                errs.append(res.get("err"))
    else:
        for r, p in enumerate(workers):
            send(p, msg_for(r))
        _mark("dispatched")
        for p in workers:
            res = recv(p)
            if not res.get("ok"):
                errs.append(res.get("err"))
    if errs:
        raise RuntimeError("worker failure:\n" + "\n".join(errs))
    _mark("workers done")

    spk = np.ndarray((T, B_FULL, NO), dtype=np.float32, buffer=shm_spk.buf)
    mem = np.ndarray((T, B_FULL, NO), dtype=np.float32, buffer=shm_mem.buf)
    return spk, mem
